# revision 3
# baseline (speedup 1.0000x reference)
"""Gated-FFN (top-1 tile-routed MoE) Trainium2 kernel.

Problem (hardcoded shapes from the spec):
  x      [B=4, T=4096, C=1024] f32
  W_gate [4, 1024], b_gate [4]
  W_up   [4096, 1024], b_up [4096]
  W_down [1024, 4096], b_down [1024]

Forward math: the straight-through gate evaluates numerically to the
one-hot argmax of the gating logits, so for a token routed to tile e:
  hidden = relu(x @ W_up[e*1024:(e+1)*1024].T + b_up[e*1024:(e+1)*1024])
  out[:, e*256:(e+1)*256] = hidden @ W_down[e*256:(e+1)*256, e*1024:(e+1)*1024].T
                            + b_down[e*256:(e+1)*256]
  all other output channels are exactly 0.

Strategy (per the sharding hint): expert-parallel routing. The host
computes the gating argmax, groups tokens by expert (4 experts), and
splits each expert's tokens across 2 of the 8 NeuronCores. Each core
runs two dense GEMMs against only its expert's weight tiles:
  GEMM1: H[ff, tok] = W_up_e @ x_shard.T     (relu + bias fused)
  GEMM2: Y[ch, tok] = W_down_e @ H           (+ bias)
Everything is laid out feature-major ([C, tok] / [ff, tok]) so both
GEMMs keep the contraction dim on SBUF partitions with no on-chip
transposes; the host pre-transposes the token shards and weights and
un-transposes the [256, cap] outputs.

Matmuls run as float32r (fp32 bits, ~fp22 multiply precision, fp32
accumulate) which streams at full PE rate (4x native fp32).
"""

import numpy as np

import concourse.bass as bass
import concourse.mybir as mybir
import concourse.tile as tile
from concourse import bacc
from concourse.bass_utils import run_bass_kernel_spmd

N_CORES = 8
NUM_TILES = 4
C = 1024
D_FF = 4096
TILE_FF = D_FF // NUM_TILES  # 1024 ff channels per expert
TILE_CH = C // NUM_TILES  # 256 output channels per expert
P = 128
KO = C // P  # 8 contraction chunks for GEMM1
MO = TILE_FF // P  # 8 ff chunks
FO = TILE_FF // P  # 8 contraction chunks for GEMM2
CHO = TILE_CH // P  # 2 output-channel chunks

F32 = mybir.dt.float32
F32R = mybir.dt.float32r

_PROGRAM_CACHE: dict = {}


def _make_chunks(cap: int) -> tuple:
    """Split cap into matmul free-dim chunks: 512s plus one optional 256."""
    assert cap % 256 == 0
    nfull, rem = divmod(cap, 512)
    return (512,) * nfull + ((256,) if rem else ())


def _build_program(chunks: tuple, repeat: int = 1):
    cap = sum(chunks)
    nc = bacc.Bacc("TRN2", target_bir_lowering=False, debug=False,
                   enable_asserts=False)
    xt = nc.dram_tensor("xt", [C, cap], F32R, kind="ExternalInput")
    w1t = nc.dram_tensor("w1t", [C, TILE_FF], F32R, kind="ExternalInput")
    w2t = nc.dram_tensor("w2t", [TILE_FF, TILE_CH], F32R, kind="ExternalInput")
    b1 = nc.dram_tensor("b1", [TILE_FF], F32, kind="ExternalInput")
    b2 = nc.dram_tensor("b2", [TILE_CH], F32, kind="ExternalInput")
    yt = nc.dram_tensor("yt", [TILE_CH, cap], F32, kind="ExternalOutput")

    xt_r = xt.ap().rearrange("(ko p) t -> p ko t", p=P)  # [128, KO, cap]
    yt_r = yt.ap().rearrange("(mo p) t -> p mo t", p=P)  # [128, CHO, cap]

    with tile.TileContext(nc) as tc:
        with (
            tc.tile_pool(name="wpool", bufs=1) as wpool,
            tc.tile_pool(name="xpool", bufs=3) as xpool,
            tc.tile_pool(name="hpool", bufs=2) as hpool,
            tc.tile_pool(name="ypool", bufs=3) as ypool,
            tc.tile_pool(name="psum", bufs=6, space="PSUM") as psum_pool,
        ):
            w1 = wpool.tile([P, KO, TILE_FF], F32R)  # w1[p,ko,f] = W_up_e.T[ko*128+p, f]
            nc.sync.dma_start(w1[:], w1t.ap().rearrange("(ko p) f -> p ko f", p=P))
            w2 = wpool.tile([P, FO, TILE_CH], F32R)  # w2[p,fo,c] = W_down_e.T[fo*128+p, c]
            nc.sync.dma_start(w2[:], w2t.ap().rearrange("(fo p) c -> p fo c", p=P))
            b1s = wpool.tile([P, MO], F32)
            nc.sync.dma_start(b1s[:], b1.ap().rearrange("(mo p) -> p mo", p=P))
            b2s = wpool.tile([P, CHO], F32)
            nc.sync.dma_start(b2s[:], b2.ap().rearrange("(mo p) -> p mo", p=P))

            for _ in range(repeat):
                off = 0
                for cw in chunks:
                    xtile = xpool.tile([P, KO, 512], F32R, tag="x")
                    nc.sync.dma_start(xtile[:, :, :cw], xt_r[:, :, off:off + cw])
                    htile = hpool.tile([P, FO, 512], F32R, tag="h")
                    for m in range(MO):
                        ps = psum_pool.tile([P, 512], F32, tag="ps")
                        for k in range(KO):
                            nc.tensor.matmul(
                                ps[:, :cw],
                                w1[:, k, m * P:(m + 1) * P],
                                xtile[:, k, :cw],
                                start=(k == 0),
                                stop=(k == KO - 1),
                            )
                        nc.scalar.activation(
                            htile[:, m, :cw], ps[:, :cw],
                            mybir.ActivationFunctionType.Relu,
                            bias=b1s[:, m:m + 1],
                        )
                    ytile = ypool.tile([P, CHO, 512], F32, tag="y")
                    for mo in range(CHO):
                        ps2 = psum_pool.tile([P, 512], F32, tag="ps")
                        for k in range(FO):
                            nc.tensor.matmul(
                                ps2[:, :cw],
                                w2[:, k, mo * P:(mo + 1) * P],
                                htile[:, k, :cw],
                                start=(k == 0),
                                stop=(k == FO - 1),
                            )
                        nc.vector.tensor_scalar_add(
                            ytile[:, mo, :cw], ps2[:, :cw], b2s[:, mo:mo + 1])
                    nc.sync.dma_start(yt_r[:, :, off:off + cw], ytile[:, :, :cw])
                    off += cw
    nc.compile()
    return nc


def _get_program(chunks: tuple, repeat: int = 1):
    key = (chunks, repeat)
    if key not in _PROGRAM_CACHE:
        _PROGRAM_CACHE[key] = _build_program(chunks, repeat)
    return _PROGRAM_CACHE[key]


def _route(xf: np.ndarray, W_gate: np.ndarray, b_gate: np.ndarray):
    """Host-side top-1 routing. Returns (expert ids, gate one-hot, per-core
    token index arrays, capacity)."""
    n = xf.shape[0]
    logits = xf.astype(np.float64) @ W_gate.astype(np.float64).T \
        + b_gate.astype(np.float64)
    expert = np.argmax(logits, axis=-1).astype(np.int64)
    gate = np.zeros((n, NUM_TILES), dtype=np.float32)
    gate[np.arange(n), expert] = 1.0

    order = np.argsort(expert, kind="stable")
    counts = np.bincount(expert, minlength=NUM_TILES)
    starts = np.concatenate(([0], np.cumsum(counts)))
    per_core_idx = []
    for e in range(NUM_TILES):
        toks = order[starts[e]:starts[e + 1]]
        half = (len(toks) + 1) // 2
        per_core_idx.append(toks[:half])
        per_core_idx.append(toks[half:])
    max_count = max(len(ix) for ix in per_core_idx)
    cap = max(256, -(-max_count // 256) * 256)
    return expert, gate, per_core_idx, cap


def _make_in_maps(xf, W_up, b_up, W_down, b_down, per_core_idx, cap):
    in_maps = []
    for core in range(N_CORES):
        e = core // 2
        idx = per_core_idx[core]
        xs = np.zeros((C, cap), dtype=np.float32)
        xs[:, :len(idx)] = xf[idx].T
        w1t = np.ascontiguousarray(
            W_up[e * TILE_FF:(e + 1) * TILE_FF, :].T).astype(np.float32)
        w2t = np.ascontiguousarray(
            W_down[e * TILE_CH:(e + 1) * TILE_CH,
                   e * TILE_FF:(e + 1) * TILE_FF].T).astype(np.float32)
        in_maps.append({
            "xt": np.ascontiguousarray(xs),
            "w1t": w1t,
            "w2t": w2t,
            "b1": np.ascontiguousarray(b_up[e * TILE_FF:(e + 1) * TILE_FF]).astype(np.float32),
            "b2": np.ascontiguousarray(b_down[e * TILE_CH:(e + 1) * TILE_CH]).astype(np.float32),
        })
    return in_maps


def kernel(x, W_gate, b_gate, W_up, b_up, W_down, b_down):
    B, T, c = x.shape
    assert c == C
    n = B * T
    xf = np.ascontiguousarray(np.asarray(x, dtype=np.float32).reshape(n, C))
    W_up = np.asarray(W_up, dtype=np.float32)
    W_down = np.asarray(W_down, dtype=np.float32)

    expert, gate, per_core_idx, cap = _route(
        xf, np.asarray(W_gate), np.asarray(b_gate))
    chunks = _make_chunks(cap)
    in_maps = _make_in_maps(xf, W_up, np.asarray(b_up), W_down,
                            np.asarray(b_down), per_core_idx, cap)

    nc = _get_program(chunks)
    res = run_bass_kernel_spmd(nc, in_maps, core_ids=list(range(N_CORES)))

    out = np.zeros((n, C), dtype=np.float32)
    for core in range(N_CORES):
        e = core // 2
        idx = per_core_idx[core]
        if len(idx) == 0:
            continue
        y = res.results[core]["yt"]  # [256, cap]
        out[idx, e * TILE_CH:(e + 1) * TILE_CH] = y[:, :len(idx)].T
    return out.reshape(B, T, C), gate.reshape(B, T, NUM_TILES).astype(np.float32)


# revision 8
# speedup vs baseline: 349.7431x; 349.7431x over previous
"""Gated-FFN (top-1 tile-routed MoE) Trainium2 kernel.

Problem (hardcoded shapes from the spec):
  x      [B=4, T=4096, C=1024] f32
  W_gate [4, 1024], b_gate [4]
  W_up   [4096, 1024], b_up [4096]
  W_down [1024, 4096], b_down [1024]

Forward math: the straight-through gate evaluates numerically to the
one-hot argmax of the gating logits, so for a token routed to tile e:
  hidden = relu(x @ W_up[e*1024:(e+1)*1024].T + b_up[e*1024:(e+1)*1024])
  out[:, e*256:(e+1)*256] = hidden @ W_down[e*256:(e+1)*256, e*1024:(e+1)*1024].T
                            + b_down[e*256:(e+1)*256]
  all other output channels are exactly 0.

Strategy (per the sharding hint): expert-parallel routing. The host
computes the gating argmax, groups tokens by expert (4 experts), and
splits each expert's tokens across 2 of the 8 NeuronCores. Each core
runs two dense GEMMs against only its expert's weight tiles:
  GEMM1: H[ff, tok] = W_up_e @ x_shard.T     (relu + bias fused)
  GEMM2: Y[ch, tok] = W_down_e @ H           (+ bias)
Everything is laid out feature-major ([C, tok] / [ff, tok]) so both
GEMMs keep the contraction dim on SBUF partitions with no on-chip
transposes; the host pre-transposes the token shards and weights and
un-transposes the [256, cap] outputs.

Matmuls run as float32r (fp32 bits, ~fp22 multiply precision, fp32
accumulate) which streams at full PE rate (4x native fp32).
"""

import numpy as np

import concourse.bass as bass
import concourse.mybir as mybir
import concourse.tile as tile
from concourse import bacc
from concourse.bass_utils import run_bass_kernel_spmd

N_CORES = 8
NUM_TILES = 4
C = 1024
D_FF = 4096
TILE_FF = D_FF // NUM_TILES  # 1024 ff channels per expert
TILE_CH = C // NUM_TILES  # 256 output channels per expert
P = 128
KO = C // P  # 8 contraction chunks for GEMM1
MO = TILE_FF // P  # 8 ff chunks
FO = TILE_FF // P  # 8 contraction chunks for GEMM2
CHO = TILE_CH // P  # 2 output-channel chunks

F32 = mybir.dt.float32
F32R = mybir.dt.float32r

_PROGRAM_CACHE: dict = {}


def _make_chunks(cap: int) -> tuple:
    """Split cap into matmul free-dim chunks, all in [256, 512] when possible.

    f32r matmuls stream 1 cycle/row at free-dim >= 256 and 4 cycles/row
    below, so chunks >= 256 make total PE cycles proportional to cap."""
    assert cap % 32 == 0
    chunks = []
    rem = cap
    while rem > 768:
        chunks.append(512)
        rem -= 512
    if rem > 512:
        chunks += [rem - 256, 256]
    elif rem:
        chunks.append(rem)
    return tuple(chunks)


def _build_program(chunks: tuple, repeat: int = 1):
    cap = sum(chunks)
    nc = bacc.Bacc("TRN2", target_bir_lowering=False, debug=False,
                   enable_asserts=False)
    xt = nc.dram_tensor("xt", [C, cap], F32R, kind="ExternalInput")
    w1t = nc.dram_tensor("w1t", [C, TILE_FF], F32R, kind="ExternalInput")
    w2t = nc.dram_tensor("w2t", [TILE_FF, TILE_CH], F32R, kind="ExternalInput")
    b1 = nc.dram_tensor("b1", [TILE_FF], F32, kind="ExternalInput")
    b2 = nc.dram_tensor("b2", [TILE_CH], F32, kind="ExternalInput")
    yt = nc.dram_tensor("yt", [TILE_CH, cap], F32, kind="ExternalOutput")

    xt_r = xt.ap().rearrange("(ko p) t -> p ko t", p=P)  # [128, KO, cap]
    yt_r = yt.ap().rearrange("(mo p) t -> p mo t", p=P)  # [128, CHO, cap]

    w1t_r = w1t.ap().rearrange("(ko p) f -> p ko f", p=P)

    with tile.TileContext(nc) as tc:
        with (
            tc.tile_pool(name="wpool", bufs=1) as wpool,
            tc.tile_pool(name="xpool", bufs=4) as xpool,
            tc.tile_pool(name="hpool", bufs=3) as hpool,
            tc.tile_pool(name="ypool", bufs=3) as ypool,
            tc.tile_pool(name="psum", bufs=8, space="PSUM") as psum_pool,
        ):
            # Weight/bias loads go on the ACT HWDGE ring (nc.scalar); token
            # loads go on the SP ring (nc.sync); result stores on SWDGE
            # (nc.gpsimd). Three independent queues so the first matmul only
            # waits for bias + w1 m-block 0 + x k-chunk 0.
            b1s = wpool.tile([P, MO], F32)
            nc.scalar.dma_start(b1s[:], b1.ap().rearrange("(mo p) -> p mo", p=P))
            b2s = wpool.tile([P, CHO], F32)
            nc.scalar.dma_start(b2s[:], b2.ap().rearrange("(mo p) -> p mo", p=P))
            w1 = wpool.tile([P, KO, TILE_FF], F32R)  # w1[p,ko,f] = W_up_e.T[ko*128+p, f]
            for m in range(MO):
                nc.scalar.dma_start(w1[:, :, m * P:(m + 1) * P],
                                    w1t_r[:, :, m * P:(m + 1) * P])
            w2 = wpool.tile([P, FO, TILE_CH], F32R)  # w2[p,fo,c] = W_down_e.T[fo*128+p, c]
            nc.scalar.dma_start(w2[:], w2t.ap().rearrange("(fo p) c -> p fo c", p=P))

            for _ in range(repeat):
                off = 0
                for cw in chunks:
                    xtile = xpool.tile([P, KO, 512], F32R, tag="x")
                    for k in range(KO):
                        nc.sync.dma_start(xtile[:, k, :cw],
                                          xt_r[:, k, off:off + cw])
                    htile = hpool.tile([P, FO, 512], F32R, tag="h")
                    for m in range(MO):
                        ps = psum_pool.tile([P, 512], F32, tag="ps")
                        for k in range(KO):
                            nc.tensor.matmul(
                                ps[:, :cw],
                                w1[:, k, m * P:(m + 1) * P],
                                xtile[:, k, :cw],
                                start=(k == 0),
                                stop=(k == KO - 1),
                            )
                        nc.scalar.activation(
                            htile[:, m, :cw], ps[:, :cw],
                            mybir.ActivationFunctionType.Relu,
                            bias=b1s[:, m:m + 1],
                        )
                    ytile = ypool.tile([P, CHO, 512], F32, tag="y")
                    for mo in range(CHO):
                        ps2 = psum_pool.tile([P, 512], F32, tag="ps")
                        for k in range(FO):
                            nc.tensor.matmul(
                                ps2[:, :cw],
                                w2[:, k, mo * P:(mo + 1) * P],
                                htile[:, k, :cw],
                                start=(k == 0),
                                stop=(k == FO - 1),
                            )
                        nc.vector.tensor_scalar_add(
                            ytile[:, mo, :cw], ps2[:, :cw], b2s[:, mo:mo + 1])
                    nc.gpsimd.dma_start(yt_r[:, :, off:off + cw],
                                        ytile[:, :, :cw])
                    off += cw
    nc.compile()
    return nc


def _get_program(chunks: tuple, repeat: int = 1):
    key = (chunks, repeat)
    if key not in _PROGRAM_CACHE:
        _PROGRAM_CACHE[key] = _build_program(chunks, repeat)
    return _PROGRAM_CACHE[key]


def _route(xf: np.ndarray, W_gate: np.ndarray, b_gate: np.ndarray):
    """Host-side top-1 routing. Returns (expert ids, gate one-hot, per-core
    token index arrays, capacity)."""
    n = xf.shape[0]
    logits = xf.astype(np.float64) @ W_gate.astype(np.float64).T \
        + b_gate.astype(np.float64)
    expert = np.argmax(logits, axis=-1).astype(np.int64)
    gate = np.zeros((n, NUM_TILES), dtype=np.float32)
    gate[np.arange(n), expert] = 1.0

    order = np.argsort(expert, kind="stable")
    counts = np.bincount(expert, minlength=NUM_TILES)
    starts = np.concatenate(([0], np.cumsum(counts)))
    per_core_idx = []
    for e in range(NUM_TILES):
        toks = order[starts[e]:starts[e + 1]]
        half = (len(toks) + 1) // 2
        per_core_idx.append(toks[:half])
        per_core_idx.append(toks[half:])
    max_count = max(len(ix) for ix in per_core_idx)
    cap = max(32, -(-max_count // 32) * 32)
    return expert, gate, per_core_idx, cap


def _make_in_maps(xf, W_up, b_up, W_down, b_down, per_core_idx, cap):
    in_maps = []
    for core in range(N_CORES):
        e = core // 2
        idx = per_core_idx[core]
        xs = np.zeros((C, cap), dtype=np.float32)
        xs[:, :len(idx)] = xf[idx].T
        w1t = np.ascontiguousarray(
            W_up[e * TILE_FF:(e + 1) * TILE_FF, :].T).astype(np.float32)
        w2t = np.ascontiguousarray(
            W_down[e * TILE_CH:(e + 1) * TILE_CH,
                   e * TILE_FF:(e + 1) * TILE_FF].T).astype(np.float32)
        in_maps.append({
            "xt": np.ascontiguousarray(xs),
            "w1t": w1t,
            "w2t": w2t,
            "b1": np.ascontiguousarray(b_up[e * TILE_FF:(e + 1) * TILE_FF]).astype(np.float32),
            "b2": np.ascontiguousarray(b_down[e * TILE_CH:(e + 1) * TILE_CH]).astype(np.float32),
        })
    return in_maps


def kernel(x, W_gate, b_gate, W_up, b_up, W_down, b_down):
    B, T, c = x.shape
    assert c == C
    n = B * T
    xf = np.ascontiguousarray(np.asarray(x, dtype=np.float32).reshape(n, C))
    W_up = np.asarray(W_up, dtype=np.float32)
    W_down = np.asarray(W_down, dtype=np.float32)

    expert, gate, per_core_idx, cap = _route(
        xf, np.asarray(W_gate), np.asarray(b_gate))
    chunks = _make_chunks(cap)
    in_maps = _make_in_maps(xf, W_up, np.asarray(b_up), W_down,
                            np.asarray(b_down), per_core_idx, cap)

    nc = _get_program(chunks)
    res = run_bass_kernel_spmd(nc, in_maps, core_ids=list(range(N_CORES)))

    out = np.zeros((n, C), dtype=np.float32)
    for core in range(N_CORES):
        e = core // 2
        idx = per_core_idx[core]
        if len(idx) == 0:
            continue
        y = res.results[core]["yt"]  # [256, cap]
        out[idx, e * TILE_CH:(e + 1) * TILE_CH] = y[:, :len(idx)].T
    return out.reshape(B, T, C), gate.reshape(B, T, NUM_TILES).astype(np.float32)


# revision 9
# speedup vs baseline: 364.0075x; 1.0408x over previous
"""Gated-FFN (top-1 tile-routed MoE) Trainium2 kernel.

Problem (hardcoded shapes from the spec):
  x      [B=4, T=4096, C=1024] f32
  W_gate [4, 1024], b_gate [4]
  W_up   [4096, 1024], b_up [4096]
  W_down [1024, 4096], b_down [1024]

Forward math: the straight-through gate evaluates numerically to the
one-hot argmax of the gating logits, so for a token routed to tile e:
  hidden = relu(x @ W_up[e*1024:(e+1)*1024].T + b_up[e*1024:(e+1)*1024])
  out[:, e*256:(e+1)*256] = hidden @ W_down[e*256:(e+1)*256, e*1024:(e+1)*1024].T
                            + b_down[e*256:(e+1)*256]
  all other output channels are exactly 0.

Strategy (per the sharding hint): expert-parallel routing. The host
computes the gating argmax, groups tokens by expert (4 experts), and
splits each expert's tokens across 2 of the 8 NeuronCores. Each core
runs two dense GEMMs against only its expert's weight tiles:
  GEMM1: H[ff, tok] = W_up_e @ x_shard.T     (relu + bias fused)
  GEMM2: Y[ch, tok] = W_down_e @ H           (+ bias)
Everything is laid out feature-major ([C, tok] / [ff, tok]) so both
GEMMs keep the contraction dim on SBUF partitions with no on-chip
transposes; the host pre-transposes the token shards and weights and
un-transposes the [256, cap] outputs.

Matmuls run as float32r (fp32 bits, ~fp22 multiply precision, fp32
accumulate) which streams at full PE rate (4x native fp32).
"""

import numpy as np

import concourse.bass as bass
import concourse.mybir as mybir
import concourse.tile as tile
from concourse import bacc
from concourse.bass_utils import run_bass_kernel_spmd

N_CORES = 8
NUM_TILES = 4
C = 1024
D_FF = 4096
TILE_FF = D_FF // NUM_TILES  # 1024 ff channels per expert
TILE_CH = C // NUM_TILES  # 256 output channels per expert
P = 128
KO = C // P  # 8 contraction chunks for GEMM1
MO = TILE_FF // P  # 8 ff chunks
FO = TILE_FF // P  # 8 contraction chunks for GEMM2
CHO = TILE_CH // P  # 2 output-channel chunks

F32 = mybir.dt.float32
F32R = mybir.dt.float32r

_PROGRAM_CACHE: dict = {}


def _make_chunks(cap: int) -> tuple:
    """Split cap into matmul free-dim chunks, all in [256, 512] when possible.

    f32r matmuls stream 1 cycle/row at free-dim >= 256 and 4 cycles/row
    below, so chunks >= 256 make total PE cycles proportional to cap."""
    assert cap % 32 == 0
    chunks = []
    rem = cap
    while rem > 768:
        chunks.append(512)
        rem -= 512
    if rem > 512:
        chunks += [rem - 256, 256]
    elif rem:
        chunks.append(rem)
    return tuple(chunks)


def _build_program(chunks: tuple, repeat: int = 1):
    cap = sum(chunks)
    nc = bacc.Bacc("TRN2", target_bir_lowering=False, debug=False,
                   enable_asserts=False)
    xt = nc.dram_tensor("xt", [C, cap], F32R, kind="ExternalInput")
    w1t = nc.dram_tensor("w1t", [C, TILE_FF], F32R, kind="ExternalInput")
    w2t = nc.dram_tensor("w2t", [TILE_FF, TILE_CH], F32R, kind="ExternalInput")
    b1 = nc.dram_tensor("b1", [TILE_FF], F32, kind="ExternalInput")
    b2 = nc.dram_tensor("b2", [TILE_CH], F32, kind="ExternalInput")
    yt = nc.dram_tensor("yt", [TILE_CH, cap], F32, kind="ExternalOutput")

    xt_r = xt.ap().rearrange("(ko p) t -> p ko t", p=P)  # [128, KO, cap]
    yt_r = yt.ap().rearrange("(mo p) t -> p mo t", p=P)  # [128, CHO, cap]

    w1t_r = w1t.ap().rearrange("(ko p) f -> p ko f", p=P)

    with tile.TileContext(nc) as tc:
        with (
            tc.tile_pool(name="wpool", bufs=1) as wpool,
            tc.tile_pool(name="xpool", bufs=5) as xpool,
            tc.tile_pool(name="hpool", bufs=3) as hpool,
            tc.tile_pool(name="ypool", bufs=3) as ypool,
            tc.tile_pool(name="psum", bufs=6, space="PSUM") as psum_pool,
            tc.tile_pool(name="psum2", bufs=2, space="PSUM") as psum2_pool,
        ):
            # Weight/bias loads go on the ACT HWDGE ring (nc.scalar); token
            # loads go on the SP ring (nc.sync); result stores on SWDGE
            # (nc.gpsimd). Three independent queues so the first matmul only
            # waits for bias + w1 m-block 0 + x k-chunk 0.
            b1s = wpool.tile([P, MO], F32)
            nc.scalar.dma_start(b1s[:], b1.ap().rearrange("(mo p) -> p mo", p=P))
            b2s = wpool.tile([P, CHO], F32)
            nc.scalar.dma_start(b2s[:], b2.ap().rearrange("(mo p) -> p mo", p=P))
            w1 = wpool.tile([P, KO, TILE_FF], F32R)  # w1[p,ko,f] = W_up_e.T[ko*128+p, f]
            for m in range(MO):
                nc.scalar.dma_start(w1[:, :, m * P:(m + 1) * P],
                                    w1t_r[:, :, m * P:(m + 1) * P])
            w2 = wpool.tile([P, FO, TILE_CH], F32R)  # w2[p,fo,c] = W_down_e.T[fo*128+p, c]
            nc.scalar.dma_start(w2[:], w2t.ap().rearrange("(fo p) c -> p fo c", p=P))

            for _ in range(repeat):
                off = 0
                for cw in chunks:
                    xtile = xpool.tile([P, KO, 512], F32R, tag="x")
                    for k in range(KO):
                        nc.sync.dma_start(xtile[:, k, :cw],
                                          xt_r[:, k, off:off + cw])
                    htile = hpool.tile([P, FO, 512], F32R, tag="h")
                    for m in range(MO):
                        ps = psum_pool.tile([P, 512], F32, tag="ps")
                        for k in range(KO):
                            nc.tensor.matmul(
                                ps[:, :cw],
                                w1[:, k, m * P:(m + 1) * P],
                                xtile[:, k, :cw],
                                start=(k == 0),
                                stop=(k == KO - 1),
                            )
                        nc.scalar.activation(
                            htile[:, m, :cw], ps[:, :cw],
                            mybir.ActivationFunctionType.Relu,
                            bias=b1s[:, m:m + 1],
                        )
                    ytile = ypool.tile([P, CHO, 512], F32, tag="y")
                    for mo in range(CHO):
                        ps2 = psum2_pool.tile([P, 512], F32, tag="ps2")
                        for k in range(FO):
                            nc.tensor.matmul(
                                ps2[:, :cw],
                                w2[:, k, mo * P:(mo + 1) * P],
                                htile[:, k, :cw],
                                start=(k == 0),
                                stop=(k == FO - 1),
                            )
                        nc.vector.tensor_scalar_add(
                            ytile[:, mo, :cw], ps2[:, :cw], b2s[:, mo:mo + 1])
                    nc.gpsimd.dma_start(yt_r[:, :, off:off + cw],
                                        ytile[:, :, :cw])
                    off += cw
    nc.compile()
    return nc


def _get_program(chunks: tuple, repeat: int = 1):
    key = (chunks, repeat)
    if key not in _PROGRAM_CACHE:
        _PROGRAM_CACHE[key] = _build_program(chunks, repeat)
    return _PROGRAM_CACHE[key]


def _route(xf: np.ndarray, W_gate: np.ndarray, b_gate: np.ndarray):
    """Host-side top-1 routing. Returns (expert ids, gate one-hot, per-core
    token index arrays, capacity)."""
    n = xf.shape[0]
    logits = xf.astype(np.float64) @ W_gate.astype(np.float64).T \
        + b_gate.astype(np.float64)
    expert = np.argmax(logits, axis=-1).astype(np.int64)
    gate = np.zeros((n, NUM_TILES), dtype=np.float32)
    gate[np.arange(n), expert] = 1.0

    order = np.argsort(expert, kind="stable")
    counts = np.bincount(expert, minlength=NUM_TILES)
    starts = np.concatenate(([0], np.cumsum(counts)))
    per_core_idx = []
    for e in range(NUM_TILES):
        toks = order[starts[e]:starts[e + 1]]
        half = (len(toks) + 1) // 2
        per_core_idx.append(toks[:half])
        per_core_idx.append(toks[half:])
    max_count = max(len(ix) for ix in per_core_idx)
    cap = max(32, -(-max_count // 32) * 32)
    return expert, gate, per_core_idx, cap


def _make_in_maps(xf, W_up, b_up, W_down, b_down, per_core_idx, cap):
    in_maps = []
    for core in range(N_CORES):
        e = core // 2
        idx = per_core_idx[core]
        xs = np.zeros((C, cap), dtype=np.float32)
        xs[:, :len(idx)] = xf[idx].T
        w1t = np.ascontiguousarray(
            W_up[e * TILE_FF:(e + 1) * TILE_FF, :].T).astype(np.float32)
        w2t = np.ascontiguousarray(
            W_down[e * TILE_CH:(e + 1) * TILE_CH,
                   e * TILE_FF:(e + 1) * TILE_FF].T).astype(np.float32)
        in_maps.append({
            "xt": np.ascontiguousarray(xs),
            "w1t": w1t,
            "w2t": w2t,
            "b1": np.ascontiguousarray(b_up[e * TILE_FF:(e + 1) * TILE_FF]).astype(np.float32),
            "b2": np.ascontiguousarray(b_down[e * TILE_CH:(e + 1) * TILE_CH]).astype(np.float32),
        })
    return in_maps


def kernel(x, W_gate, b_gate, W_up, b_up, W_down, b_down):
    B, T, c = x.shape
    assert c == C
    n = B * T
    xf = np.ascontiguousarray(np.asarray(x, dtype=np.float32).reshape(n, C))
    W_up = np.asarray(W_up, dtype=np.float32)
    W_down = np.asarray(W_down, dtype=np.float32)

    expert, gate, per_core_idx, cap = _route(
        xf, np.asarray(W_gate), np.asarray(b_gate))
    chunks = _make_chunks(cap)
    in_maps = _make_in_maps(xf, W_up, np.asarray(b_up), W_down,
                            np.asarray(b_down), per_core_idx, cap)

    nc = _get_program(chunks)
    res = run_bass_kernel_spmd(nc, in_maps, core_ids=list(range(N_CORES)))

    out = np.zeros((n, C), dtype=np.float32)
    for core in range(N_CORES):
        e = core // 2
        idx = per_core_idx[core]
        if len(idx) == 0:
            continue
        y = res.results[core]["yt"]  # [256, cap]
        out[idx, e * TILE_CH:(e + 1) * TILE_CH] = y[:, :len(idx)].T
    return out.reshape(B, T, C), gate.reshape(B, T, NUM_TILES).astype(np.float32)
